# revision 14
# baseline (speedup 1.0000x reference)
"""VQ half-integer 4-bit codebook quantize + nibble-pack on 8 Trainium2 NeuronCores.

Reference semantics (see problem): for x in X[4096,4096,1], grid = (-8..7)+0.5:
  idx    = argmax_k (2*x*g_k - g_k^2)        (nearest grid point, f32 score argmax)
  vals   = grid[idx]
  packed = nibble-pack of idx, 8 per int32, shifts [28,12,24,8,20,4,16,0]
  dequant= grid[unpack(packed)] == vals      (identity: pack/unpack is lossless)

Device computes, per element: idx = RNE(min(x, 7.9) + 7.5) converted to uint16,
whose saturating converter clamps negatives to 0 (= clamp(floor(x)+8, 0, 15) away
from integer decision boundaries, one dual-op DVE tensor_scalar); vals = idx-7.5
and the int8 idx container on ScalarE; the nibble pack runs on the vector engine
as two u16 half-word shift/or chains whose finals write interleaved u16 slots,
assembling the little-endian int32 packed words directly in SBUF. Rows are
sharded 512/core across 8 cores; loads ride the SP DMA ring, stores the ACT ring.

Outputs leave the device in exact compact containers and are widened during the
host-side unshard: vals as bf16 (half-integers <= 7.5 are exact in bf16), idx as
int8 (values 0..15), packed as int32. dequant is a bitwise copy of vals (provable
identity). Elements within 1e-4 of an integer (the argmax decision boundaries,
~1e-7 of the data) are recomputed on the host with the exact per-op f32 score
argmax so the result is bit-exact against the reference's own f32 rounding.
"""

import numpy as np

R, C = 4096, 4096
NCORES = 8
RPC = R // NCORES          # rows per core
P = 128                    # SBUF partitions
T = 4096                   # chunk free-dim size
NCHUNK = (RPC * C) // (P * T)   # 8
# fat chunk layout, in BYTES per row: vals bf16 (2T) | idx i8 (T) | packed i32 (T/2)
TFB = 2 * T + T + T // 2
SHIFTS = np.array([28, 12, 24, 8, 20, 4, 16, 0], dtype=np.int32)

_prog_cache = {}


def _build_program():
    import concourse.bacc as bacc
    import concourse.mybir as mybir
    from concourse.tile import TileContext

    nc = bacc.Bacc("TRN2", target_bir_lowering=False, debug=False)
    x = nc.dram_tensor("x", [RPC, C], mybir.dt.float32, kind="ExternalInput")
    # chunk-major fat output (int8 container): chunk k = rows [k*128,(k+1)*128)
    out = nc.dram_tensor("out", [NCHUNK * P, TFB], mybir.dt.int8, kind="ExternalOutput")
    nsub = C // T  # column sub-chunks per 128-row block
    x_r = x.ap().rearrange("(n p) (c t) -> n c p t", p=P, t=T)
    o_r = out.ap().rearrange("(k p) m -> k p m", p=P)

    Alu = mybir.AluOpType
    Act = mybir.ActivationFunctionType

    def stt_u16(out_ap, in0_ap, shift, in1_ap):
        # (in0 << shift) | in1 on u16; patch the imm to a matching-int dtype
        inst = nc.vector.scalar_tensor_tensor(
            out_ap, in0_ap, float(shift), in1_ap,
            Alu.logical_shift_left, Alu.bitwise_or)
        raw = inst.ins
        raw.ins = [raw.ins[0],
                   mybir.ImmediateValue(dtype=mybir.dt.uint16, value=int(shift)),
                   raw.ins[2]]
        return inst

    with TileContext(nc) as tc:
        with tc.tile_pool(name="xpool", bufs=NCHUNK) as xpool, \
             tc.tile_pool(name="tpool", bufs=2) as tpool, \
             tc.tile_pool(name="spool", bufs=2) as spool, \
             tc.tile_pool(name="pool", bufs=3) as pool:
            xts = []
            for k in range(NCHUNK):
                nb, cm = divmod(k, nsub)
                xt = xpool.tile([P, T], mybir.dt.float32, tag="x")
                nc.sync.dma_start(xt[:], x_r[nb, cm])
                xts.append(xt)
            for k in range(NCHUNK):
                xt = xts[k]
                u16 = tpool.tile([P, T], mybir.dt.uint16, tag="u16")
                fat = pool.tile([P, TFB], mybir.dt.int8, tag="fat")
                vals = fat[:, 0:2 * T].bitcast(mybir.dt.bfloat16)
                idx8 = fat[:, 2 * T:3 * T]
                pk16 = fat[:, 3 * T:].bitcast(mybir.dt.uint16)   # [P, T/4]

                # idx = RNE(min(x,7.9) + 7.5), saturating u16 clamps negatives to 0
                # == clamp(floor(x)+8, 0, 15) away from decision boundaries (DVE)
                nc.vector.tensor_scalar(u16[:], xt[:], 7.9, 7.5, Alu.min, Alu.add)
                # vals(bf16) = idx - 7.5   (ACT; half-integers are exact in bf16)
                nc.scalar.activation(vals, u16[:], Act.Copy, bias=-7.5, scale=1.0)
                # idx8 = idx (int8 container)   (ACT copy; values 0..15 exact)
                nc.scalar.activation(idx8, u16[:], Act.Copy, bias=0.0, scale=1.0)

                # packed, as two u16 halves assembled in-place (little-endian):
                #   lo16 = i1<<12 | i3<<8 | i5<<4 | i7   -> even u16 slots
                #   hi16 = i0<<12 | i2<<8 | i4<<4 | i6   -> odd  u16 slots
                ph = [u16[:, p:T:8] for p in range(8)]
                for base, dst in ((1, pk16[:, 0:T // 4:2]), (0, pk16[:, 1:T // 4:2])):
                    s = spool.tile([P, T // 8], mybir.dt.uint16, tag="s")
                    stt_u16(s[:], ph[base + 4], 4, ph[base + 6])
                    stt_u16(s[:], ph[base + 2], 8, s[:])
                    stt_u16(dst, ph[base], 12, s[:])

                nc.scalar.dma_start(o_r[k], fat[:])

    nc.compile()
    return nc


def _get_program():
    if "nc" not in _prog_cache:
        _prog_cache["nc"] = _build_program()
    return _prog_cache["nc"]


def _host_boundary_fixup(x2, g, vals, idx, packed):
    """Recompute elements near integer decision boundaries with the exact
    per-op f32 score argmax (matches the reference bit-for-bit)."""
    near = np.abs(x2 - np.rint(x2)) <= np.float32(1e-4)
    rr, cc = np.nonzero(near)
    if rr.size == 0:
        return
    xm = x2[rr, cc].astype(np.float32)
    scores = (np.float32(2.0) * (xm[:, None] * g[None, :])).astype(np.float32) \
        - (g * g).astype(np.float32)
    ifix = np.argmax(scores, axis=1).astype(np.int32)
    idx[rr, cc] = ifix
    vals[rr, cc] = g[ifix]
    # re-pack affected words
    gi = cc // 8
    flat = np.unique(rr.astype(np.int64) * (C // 8) + gi)
    pr = (flat // (C // 8)).astype(np.int64)
    pg = (flat % (C // 8)).astype(np.int64)
    grp = idx[pr[:, None], (pg * 8)[:, None] + np.arange(8)[None, :]]
    w = np.bitwise_or.reduce(
        (grp.astype(np.uint32) << SHIFTS.astype(np.uint32)[None, :]), axis=1)
    packed[pr, pg] = w.view(np.int32)


def kernel(X, grid):
    import ml_dtypes
    from concourse.bass_utils import run_bass_kernel_spmd

    x2 = np.ascontiguousarray(np.asarray(X, dtype=np.float32).reshape(R, C))
    g = np.asarray(grid, dtype=np.float32).reshape(-1)

    nc = _get_program()
    in_maps = [{"x": x2[c * RPC:(c + 1) * RPC]} for c in range(NCORES)]
    res = run_bass_kernel_spmd(nc, in_maps, core_ids=list(range(NCORES)))

    nsub = C // T
    nblk = RPC // P
    vals_l, idx_l, pk_l = [], [], []
    for r in res.results:
        f5 = r["out"].reshape(nblk, nsub, P, TFB)   # [nb, cm, p, bytes]
        vb = f5[..., :2 * T].view(ml_dtypes.bfloat16)          # [nb, cm, p, T]
        ib = f5[..., 2 * T:3 * T]                              # int8
        pb = np.ascontiguousarray(f5[..., 3 * T:]).view(np.int32)
        vals_l.append(vb.transpose(0, 2, 1, 3).reshape(RPC, C))
        idx_l.append(ib.transpose(0, 2, 1, 3).reshape(RPC, C))
        pk_l.append(pb.transpose(0, 2, 1, 3).reshape(RPC, C // 8))
    vals = np.concatenate(vals_l, axis=0).astype(np.float32)   # exact widening
    idx = np.concatenate(idx_l, axis=0).astype(np.int32)       # exact widening
    packed = np.ascontiguousarray(np.concatenate(pk_l, axis=0))

    _host_boundary_fixup(x2, g, vals, idx, packed)

    dequant = vals.copy()  # pack/unpack of in-range nibbles is the identity
    return (vals.reshape(R, C, 1), idx, packed, dequant.reshape(R, C, 1))


# revision 21
# speedup vs baseline: 1.0803x; 1.0803x over previous
"""VQ half-integer 4-bit codebook quantize + nibble-pack on 8 Trainium2 NeuronCores.

Reference semantics (see problem): for x in X[4096,4096,1], grid = (-8..7)+0.5:
  idx    = argmax_k (2*x*g_k - g_k^2)        (nearest grid point, f32 score argmax)
  vals   = grid[idx]
  packed = nibble-pack of idx, 8 per int32, shifts [28,12,24,8,20,4,16,0]
  dequant= grid[unpack(packed)] == vals      (identity: pack/unpack is lossless)

Device computes, per element: idx = RNE(min(x, 7.9) + 7.5) converted to uint16,
whose saturating converter clamps negatives to 0 (= clamp(floor(x)+8, 0, 15) away
from integer decision boundaries, one dual-op DVE tensor_scalar); vals = idx-7.5
and the int8 idx container on ScalarE; the nibble pack runs on the vector engine
as two u16 half-word shift/or chains whose finals write interleaved u16 slots,
assembling the little-endian int32 packed words directly in SBUF. Rows are
sharded 512/core across 8 cores; loads ride the SP DMA ring, stores the ACT ring.

Outputs leave the device in exact compact containers and are widened during the
host-side unshard: vals as bf16 (half-integers <= 7.5 are exact in bf16), idx as
int8 (values 0..15), packed as int32. dequant is a bitwise copy of vals (provable
identity). Elements within 1e-4 of an integer (the argmax decision boundaries,
~1e-7 of the data) are recomputed on the host with the exact per-op f32 score
argmax so the result is bit-exact against the reference's own f32 rounding.
"""

import numpy as np

R, C = 4096, 4096
NCORES = 8
RPC = R // NCORES          # rows per core
P = 128                    # SBUF partitions
T = 2048                   # chunk free-dim size
NCHUNK = (RPC * C) // (P * T)   # 8
# fat chunk layout, in BYTES per row: vals bf16 (2T) | idx i8 (T) | packed i32 (T/2)
TFB = 2 * T + T + T // 2
SHIFTS = np.array([28, 12, 24, 8, 20, 4, 16, 0], dtype=np.int32)

_prog_cache = {}


def _build_program():
    import concourse.bacc as bacc
    import concourse.mybir as mybir
    from concourse.tile import TileContext

    nc = bacc.Bacc("TRN2", target_bir_lowering=False, debug=False)
    x = nc.dram_tensor("x", [RPC, C], mybir.dt.float32, kind="ExternalInput")
    # chunk-major fat output (int8 container): chunk k = rows [k*128,(k+1)*128)
    out = nc.dram_tensor("out", [NCHUNK * P, TFB], mybir.dt.int8, kind="ExternalOutput")
    nsub = C // T  # column sub-chunks per 128-row block
    x_r = x.ap().rearrange("(n p) (c t) -> n c p t", p=P, t=T)
    o_r = out.ap().rearrange("(k p) m -> k p m", p=P)

    Alu = mybir.AluOpType
    Act = mybir.ActivationFunctionType

    def stt_u16(out_ap, in0_ap, shift, in1_ap):
        # (in0 << shift) | in1 on u16; patch the imm to a matching-int dtype
        inst = nc.vector.scalar_tensor_tensor(
            out_ap, in0_ap, float(shift), in1_ap,
            Alu.logical_shift_left, Alu.bitwise_or)
        raw = inst.ins
        raw.ins = [raw.ins[0],
                   mybir.ImmediateValue(dtype=mybir.dt.uint16, value=int(shift)),
                   raw.ins[2]]
        return inst

    with TileContext(nc) as tc:
        with tc.tile_pool(name="xpool", bufs=NCHUNK) as xpool, \
             tc.tile_pool(name="tpool", bufs=2) as tpool, \
             tc.tile_pool(name="spool", bufs=2) as spool, \
             tc.tile_pool(name="pool", bufs=3) as pool:
            xts = []
            for k in range(NCHUNK):
                nb, cm = divmod(k, nsub)
                xt = xpool.tile([P, T], mybir.dt.float32, tag="x")
                nc.sync.dma_start(xt[:], x_r[nb, cm])
                xts.append(xt)
            for k in range(NCHUNK):
                xt = xts[k]
                u16 = tpool.tile([P, T], mybir.dt.uint16, tag="u16")
                fat = pool.tile([P, TFB], mybir.dt.int8, tag="fat")
                vals = fat[:, 0:2 * T].bitcast(mybir.dt.bfloat16)
                idx8 = fat[:, 2 * T:3 * T]
                pk16 = fat[:, 3 * T:].bitcast(mybir.dt.uint16)   # [P, T/4]

                # idx = RNE(min(x,7.9) + 7.5), saturating u16 clamps negatives to 0
                # == clamp(floor(x)+8, 0, 15) away from decision boundaries (DVE)
                nc.vector.tensor_scalar(u16[:], xt[:], 7.9, 7.5, Alu.min, Alu.add)
                # vals(bf16) = idx - 7.5   (ACT; half-integers are exact in bf16)
                nc.scalar.activation(vals, u16[:], Act.Copy, bias=-7.5, scale=1.0)
                # idx8 = idx (int8 container; values 0..15 exact). ACT normally;
                # for the last chunk DVE computes it straight from x (saturating
                # u8) - ACT is the backlogged engine on the final critical tail.
                if k == NCHUNK - 1:
                    nc.vector.tensor_scalar(idx8.bitcast(mybir.dt.uint8), xt[:],
                                            7.9, 7.5, Alu.min, Alu.add)
                else:
                    nc.scalar.activation(idx8, u16[:], Act.Copy, bias=0.0, scale=1.0)

                # packed, as two u16 halves assembled in-place (little-endian):
                #   lo16 = i1<<12 | i3<<8 | i5<<4 | i7   -> even u16 slots
                #   hi16 = i0<<12 | i2<<8 | i4<<4 | i6   -> odd  u16 slots
                ph = [u16[:, p:T:8] for p in range(8)]
                for base, dst in ((1, pk16[:, 0:T // 4:2]), (0, pk16[:, 1:T // 4:2])):
                    s = spool.tile([P, T // 8], mybir.dt.uint16, tag="s")
                    stt_u16(s[:], ph[base + 4], 4, ph[base + 6])
                    stt_u16(s[:], ph[base + 2], 8, s[:])
                    stt_u16(dst, ph[base], 12, s[:])

                nc.scalar.dma_start(o_r[k], fat[:])

    nc.compile()
    return nc


def _get_program():
    if "nc" not in _prog_cache:
        _prog_cache["nc"] = _build_program()
    return _prog_cache["nc"]


def _host_boundary_fixup(x2, g, vals, idx, packed):
    """Recompute elements near integer decision boundaries with the exact
    per-op f32 score argmax (matches the reference bit-for-bit)."""
    near = np.abs(x2 - np.rint(x2)) <= np.float32(1e-4)
    rr, cc = np.nonzero(near)
    if rr.size == 0:
        return
    xm = x2[rr, cc].astype(np.float32)
    scores = (np.float32(2.0) * (xm[:, None] * g[None, :])).astype(np.float32) \
        - (g * g).astype(np.float32)
    ifix = np.argmax(scores, axis=1).astype(np.int32)
    idx[rr, cc] = ifix
    vals[rr, cc] = g[ifix]
    # re-pack affected words
    gi = cc // 8
    flat = np.unique(rr.astype(np.int64) * (C // 8) + gi)
    pr = (flat // (C // 8)).astype(np.int64)
    pg = (flat % (C // 8)).astype(np.int64)
    grp = idx[pr[:, None], (pg * 8)[:, None] + np.arange(8)[None, :]]
    w = np.bitwise_or.reduce(
        (grp.astype(np.uint32) << SHIFTS.astype(np.uint32)[None, :]), axis=1)
    packed[pr, pg] = w.view(np.int32)


def kernel(X, grid):
    import ml_dtypes
    from concourse.bass_utils import run_bass_kernel_spmd

    x2 = np.ascontiguousarray(np.asarray(X, dtype=np.float32).reshape(R, C))
    g = np.asarray(grid, dtype=np.float32).reshape(-1)

    nc = _get_program()
    in_maps = [{"x": x2[c * RPC:(c + 1) * RPC]} for c in range(NCORES)]
    res = run_bass_kernel_spmd(nc, in_maps, core_ids=list(range(NCORES)))

    nsub = C // T
    nblk = RPC // P
    vals_l, idx_l, pk_l = [], [], []
    for r in res.results:
        f5 = r["out"].reshape(nblk, nsub, P, TFB)   # [nb, cm, p, bytes]
        vb = f5[..., :2 * T].view(ml_dtypes.bfloat16)          # [nb, cm, p, T]
        ib = f5[..., 2 * T:3 * T]                              # int8
        pb = np.ascontiguousarray(f5[..., 3 * T:]).view(np.int32)
        vals_l.append(vb.transpose(0, 2, 1, 3).reshape(RPC, C))
        idx_l.append(ib.transpose(0, 2, 1, 3).reshape(RPC, C))
        pk_l.append(pb.transpose(0, 2, 1, 3).reshape(RPC, C // 8))
    vals = np.concatenate(vals_l, axis=0).astype(np.float32)   # exact widening
    idx = np.concatenate(idx_l, axis=0).astype(np.int32)       # exact widening
    packed = np.ascontiguousarray(np.concatenate(pk_l, axis=0))

    _host_boundary_fixup(x2, g, vals, idx, packed)

    dequant = vals.copy()  # pack/unpack of in-range nibbles is the identity
    return (vals.reshape(R, C, 1), idx, packed, dequant.reshape(R, C, 1))
